# revision 8
# baseline (speedup 1.0000x reference)
"""MoE (top-2 of 8 experts) Trainium2 kernel, 8-core data-parallel.

Sharding: tokens are split 8 ways (1024 tokens/core); every core holds all
8 experts' weights and computes its tokens' full MoE output. No collectives.

Per-core program (all on device):
  - fp32 router: logits = x @ Wg + bg, softmax, top-2 mask -> gates g [T, E]
  - bf16 FFN per expert with fp32 PSUM accumulation, exact-erf Gelu on ACT
  - gated accumulate into fp32 acc, DMA out

Layout notes:
  - x is passed transposed (feature-major) so both router and layer-1
    matmuls use natural layouts with zero on-device transposes.
  - biases: b1 is applied via ACT per-partition bias (h on partitions);
    b2 and bg are added with a K=1 ones-row matmul into the PSUM group.
"""

import os
import sys

for _p in ("/opt/trn_rl_repo", "/root/.axon_site/_ro/trn_rl_repo"):
    if os.path.isdir(_p) and _p not in sys.path:
        sys.path.insert(0, _p)

import numpy as np
import ml_dtypes

import concourse.bass as bass
import concourse.bacc as bacc
import concourse.tile as tile
from concourse import mybir
from concourse.bass_utils import run_bass_kernel_spmd

F32 = mybir.dt.float32
BF16 = mybir.dt.bfloat16
AF = mybir.ActivationFunctionType
ALU = mybir.AluOpType
BFNP = ml_dtypes.bfloat16

D = 1024      # in_features
H = 4096      # hidden
E = 8         # experts
TOPK = 2
N_CORES = 8
T = 1024      # tokens per core
NT = T // 128   # token tiles per core (8)
ND = D // 128   # feature tiles (8)
NH = H // 128   # hidden tiles (32)

DEBUG_ROUTER = False   # add logits/gates as extra outputs


def _emit_router(nc, tc, pools, xt32, wg_t, bg_t, ones32, g):
    """Router: fp32 logits -> top-2 gate weights g [128, NT*E]."""
    spool = pools["router_sbuf"]
    rpsum = pools["router_psum"]
    for tt in range(NT):
        ps = rpsum.tile([128, E], F32)
        for dt in range(ND):
            nc.tensor.matmul(
                ps[:],
                lhsT=xt32[:, dt * T + tt * 128: dt * T + (tt + 1) * 128],
                rhs=wg_t[:, dt * E:(dt + 1) * E],
                start=(dt == 0),
                stop=False,
            )
        # += ones(128).T @ bg  (adds bg to every token row)
        nc.tensor.matmul(ps[:], lhsT=ones32[:], rhs=bg_t[:], start=False, stop=True)

        logit = spool.tile([128, E], F32, tag="logit")
        nc.scalar.copy(logit[:], ps[:])
        m8 = spool.tile([128, 8], F32, tag="m8")
        nc.vector.max(m8[:], logit[:])
        negm = spool.tile([128, 1], F32, tag="negm")
        nc.vector.tensor_scalar_mul(negm[:], m8[:, 0:1], -1.0)
        p = spool.tile([128, E], F32, tag="p")
        nc.scalar.activation(p[:], logit[:], AF.Exp, bias=negm[:, 0:1])
        s = spool.tile([128, 1], F32, tag="s")
        nc.vector.reduce_sum(s[:], p[:], axis=mybir.AxisListType.X)
        r = spool.tile([128, 1], F32, tag="r")
        nc.vector.reciprocal(r[:], s[:])
        keep = spool.tile([128, E], F32, tag="keep")
        nc.vector.tensor_scalar(
            keep[:], logit[:], m8[:, 1:2], None, op0=ALU.is_ge
        )
        # g = (p * 1/s) * keep
        nc.vector.scalar_tensor_tensor(
            out=g[:, tt * E:(tt + 1) * E],
            in0=p[:],
            scalar=r[:, 0:1],
            in1=keep[:],
            op0=ALU.mult,
            op1=ALU.mult,
        )
        if DEBUG_ROUTER:
            nc.sync.dma_start(
                nc.dbg_logits.ap()[tt * 128:(tt + 1) * 128, :], logit[:]
            )


def _emit_dense_moe(nc, tc, io):
    """Dense MoE: every expert computes all tokens; gates mask the combine."""
    from contextlib import ExitStack

    with ExitStack() as ctx:
        const = ctx.enter_context(tc.tile_pool(name="const", bufs=1))
        xpool = ctx.enter_context(tc.tile_pool(name="x", bufs=1))
        gpool = ctx.enter_context(tc.tile_pool(name="g", bufs=1))
        accpool = ctx.enter_context(tc.tile_pool(name="acc", bufs=1))
        hpool = ctx.enter_context(tc.tile_pool(name="hT", bufs=2))
        w1pool = ctx.enter_context(tc.tile_pool(name="w1s", bufs=3))
        w2pool = ctx.enter_context(tc.tile_pool(name="w2s", bufs=3))
        psum1 = ctx.enter_context(tc.tile_pool(name="psum1", bufs=2, space="PSUM"))
        psum2 = ctx.enter_context(tc.tile_pool(name="psum2", bufs=4, space="PSUM"))

        # ---- constants / persistent tiles ----
        ones32 = const.tile([1, 128], F32)
        nc.vector.memset(ones32[:], 1.0)
        ones16 = const.tile([1, 128], BF16)
        nc.vector.memset(ones16[:], 1.0)

        wg_t = const.tile([128, ND * E], F32)      # [p, dt*E+e]
        nc.sync.dma_start(
            wg_t[:].rearrange("p (a e) -> p a e", e=E),
            io["Wg"].ap().rearrange("(a p) e -> p a e", p=128),
        )
        bg_t = const.tile([1, E], F32)
        nc.sync.dma_start(bg_t[:], io["bg"].ap())
        b1c_t = const.tile([128, E * NH], F32)     # [p, e*NH+i]
        nc.sync.dma_start(
            b1c_t[:].rearrange("p (e i) -> p e i", i=NH),
            io["b1c"].ap().rearrange("e p i -> p e i"),
        )
        b2r_t = const.tile([1, E * D], BF16)       # [0, e*D+d]
        nc.sync.dma_start(
            b2r_t[:].rearrange("o (e d) -> o e d", d=D),
            io["b2r"].ap().rearrange("e o d -> o e d"),
        )

        xt32 = xpool.tile([128, ND * T], F32)      # [p, dt*T+t] = xT[dt*128+p, t]
        nc.sync.dma_start(
            xt32[:].rearrange("p (a t) -> p a t", t=T),
            io["xT32"].ap().rearrange("(a p) t -> p a t", p=128),
        )
        xt16 = xpool.tile([128, ND * T], BF16)
        nc.sync.dma_start(
            xt16[:].rearrange("p (a t) -> p a t", t=T),
            io["xT16"].ap().rearrange("(a p) t -> p a t", p=128),
        )

        g = gpool.tile([128, NT * E], F32)

        with tc.tile_pool(name="router_sbuf", bufs=2) as rs, tc.tile_pool(
            name="router_psum", bufs=2, space="PSUM"
        ) as rp:
            pools = {"router_sbuf": rs, "router_psum": rp}
            _emit_router(nc, tc, pools, xt32, wg_t, bg_t, ones32, g)

        acc = accpool.tile([128, NT * D], F32)     # [p, mt*D+d]
        nc.vector.memset(acc[:], 0.0)

        W1 = io["W1"].ap()   # [E, D, H] bf16
        W2 = io["W2"].ap()   # [E, H, D] bf16

        HALF = T // 2        # 512 tokens per half
        MT_HALF = HALF // 128  # 4 token tiles per half

        for e in range(E):
            for half in range(2):
                # ---- layer 1: hT[h, tok_half] = gelu(W1[e].T @ x + b1) ----
                hT = hpool.tile([128, NH * HALF], BF16)   # [p, ht*HALF + t]
                for ht in range(NH):
                    w1s = w1pool.tile([128, ND * 128], BF16)  # [p, dt*128+j]
                    nc.sync.dma_start(
                        w1s[:].rearrange("p (a j) -> p a j", j=128),
                        W1[e]
                        .rearrange("(a p) h -> p a h", p=128)[
                            :, :, ht * 128:(ht + 1) * 128
                        ],
                    )
                    ps = psum1.tile([128, HALF], F32)
                    for dt in range(ND):
                        nc.tensor.matmul(
                            ps[:],
                            lhsT=w1s[:, dt * 128:(dt + 1) * 128],
                            rhs=xt16[
                                :, dt * T + half * HALF: dt * T + (half + 1) * HALF
                            ],
                            start=(dt == 0),
                            stop=(dt == ND - 1),
                        )
                    nc.scalar.activation(
                        hT[:, ht * HALF:(ht + 1) * HALF],
                        ps[:],
                        AF.Gelu,
                        bias=b1c_t[:, e * NH + ht: e * NH + ht + 1],
                    )

                # ---- layer 2 + gated accumulate ----
                for db in range(2):   # d blocks of 512
                    pss = [
                        psum2.tile([128, 512], F32, name=f"ps2_{mt}", tag="ps2")
                        for mt in range(MT_HALF)
                    ]
                    for ht in range(NH):
                        w2s = w2pool.tile([128, 512], BF16)
                        nc.sync.dma_start(
                            w2s[:],
                            W2[e][
                                ht * 128:(ht + 1) * 128,
                                db * 512:(db + 1) * 512,
                            ],
                        )
                        for mt in range(MT_HALF):
                            nc.tensor.matmul(
                                pss[mt][:],
                                lhsT=hT[
                                    :, ht * HALF + mt * 128: ht * HALF + (mt + 1) * 128
                                ],
                                rhs=w2s[:],
                                start=(ht == 0),
                                stop=False,
                            )
                    for mt in range(MT_HALF):
                        nc.tensor.matmul(
                            pss[mt][:],
                            lhsT=ones16[:],
                            rhs=b2r_t[:, e * D + db * 512: e * D + (db + 1) * 512],
                            start=False,
                            stop=True,
                        )
                        gmt = half * MT_HALF + mt
                        sl = slice(gmt * D + db * 512, gmt * D + (db + 1) * 512)
                        nc.vector.scalar_tensor_tensor(
                            out=acc[:, sl],
                            in0=pss[mt][:],
                            scalar=g[:, gmt * E + e: gmt * E + e + 1],
                            in1=acc[:, sl],
                            op0=ALU.mult,
                            op1=ALU.add,
                        )

        # ---- store ----
        out = io["out"].ap()   # [T, D] f32
        for mt in range(NT):
            nc.sync.dma_start(
                out[mt * 128:(mt + 1) * 128, :],
                acc[:, mt * D:(mt + 1) * D],
            )


CCHUNK = 384   # max selected-token slots per expert chunk (keeps PSUM <= 8 banks)


def route_capacities(inputs):
    """Host router (numpy) -> per-expert capacity chunks (compile-time shapes).

    Only shapes are derived here; the device recomputes the routing itself.
    """
    x = np.asarray(inputs["x"], np.float32).reshape(-1, D)
    logits = x @ np.asarray(inputs["Wg"], np.float32) + np.asarray(
        inputs["bg"], np.float32
    )
    srt2 = np.sort(logits, -1)[:, -2:-1]
    sel = logits >= srt2               # [N, E] top-2 membership
    counts = sel.reshape(N_CORES, T, E).sum(axis=1)   # [cores, E]
    caps = counts.max(axis=0)
    chunks = []
    for e in range(E):
        cap = int(np.ceil((int(caps[e]) + 8) / 64.0) * 64)
        s0 = 0
        while cap > 0:
            c = min(cap, CCHUNK)
            chunks.append((e, s0, c))
            s0 += c
            cap -= c
    return tuple(chunks)


def _emit_sparse_moe(nc, tc, io, chunks):
    """Sparse MoE: gather top-2 tokens per expert via one-hot matmuls,
    run the FFN only on selected slots, scatter gated rows to (token, rank)
    pair slots in DRAM, then combine pairs."""
    from contextlib import ExitStack

    pairs = nc.dram_tensor("pairs", [2 * T, D], F32)

    with ExitStack() as ctx:
        const = ctx.enter_context(tc.tile_pool(name="const", bufs=1))
        xpool = ctx.enter_context(tc.tile_pool(name="x", bufs=1))
        rout = ctx.enter_context(tc.tile_pool(name="rout", bufs=1))
        ppool = ctx.enter_context(tc.tile_pool(name="P16", bufs=2))
        selpool = ctx.enter_context(tc.tile_pool(name="sel", bufs=2))
        hpool = ctx.enter_context(tc.tile_pool(name="hTs", bufs=2))
        w1pool = ctx.enter_context(tc.tile_pool(name="w1s", bufs=3))
        w2pool = ctx.enter_context(tc.tile_pool(name="w2s", bufs=3))
        epool = ctx.enter_context(tc.tile_pool(name="eo", bufs=3))
        mpool = ctx.enter_context(tc.tile_pool(name="meta", bufs=2))
        cpool = ctx.enter_context(tc.tile_pool(name="comb", bufs=2))
        b2pool = ctx.enter_context(tc.tile_pool(name="b2e", bufs=2))
        psA = ctx.enter_context(tc.tile_pool(name="psA", bufs=2, space="PSUM"))
        psB = ctx.enter_context(tc.tile_pool(name="psB", bufs=6, space="PSUM"))

        # ---- constants ----
        ones32 = const.tile([1, 128], F32)
        nc.vector.memset(ones32[:], 1.0)
        ones16 = const.tile([1, 128], BF16)
        nc.vector.memset(ones16[:], 1.0)
        ones128 = const.tile([128, 128], F32)
        nc.vector.memset(ones128[:], 1.0)

        iota_i = const.tile([128, CCHUNK], mybir.dt.int32)
        nc.gpsimd.iota(iota_i[:], pattern=[[1, CCHUNK]], base=0, channel_multiplier=0)
        iota_f = const.tile([128, CCHUNK], F32)
        nc.vector.tensor_copy(iota_f[:], iota_i[:])

        lt_i = const.tile([128, 128], mybir.dt.int32)   # j - p
        nc.gpsimd.iota(lt_i[:], pattern=[[1, 128]], base=0, channel_multiplier=-1)
        lt_f = const.tile([128, 128], F32)
        nc.vector.tensor_copy(lt_f[:], lt_i[:])
        lstrict = const.tile([128, 128], F32)           # 1 iff p < j
        nc.vector.tensor_scalar(lstrict[:], lt_f[:], 0.0, None, op0=ALU.is_gt)

        tgl_i = const.tile([128, NT], mybir.dt.int32)   # 1 + p + 128*tt
        nc.gpsimd.iota(tgl_i[:], pattern=[[128, NT]], base=1, channel_multiplier=1)
        tgl_f = const.tile([128, NT], F32)
        nc.vector.tensor_copy(tgl_f[:], tgl_i[:])

        wg_t = const.tile([128, ND * E], F32)
        nc.sync.dma_start(
            wg_t[:].rearrange("p (a e) -> p a e", e=E),
            io["Wg"].ap().rearrange("(a p) e -> p a e", p=128),
        )
        bg_t = const.tile([1, E], F32)
        nc.sync.dma_start(bg_t[:], io["bg"].ap())
        b1c_t = const.tile([128, E * NH], F32)
        nc.sync.dma_start(
            b1c_t[:].rearrange("p (e i) -> p e i", i=NH),
            io["b1c"].ap().rearrange("e p i -> p e i"),
        )
        xtok = xpool.tile([128, NT * D], BF16)   # [p, tt*D+d] = x[tt*128+p, d]
        nc.sync.dma_start(
            xtok[:].rearrange("p (a d) -> p a d", d=D),
            io["x16"].ap().rearrange("(a p) d -> p a d", p=128),
        )

        # ---- router (fp32) + routing metadata ----
        g = rout.tile([128, NT * E], F32)
        keepT = rout.tile([128, NT * E], F32)
        slotv = rout.tile([128, NT * E], F32)
        pos = rout.tile([128, NT * E], F32)

        with tc.tile_pool(name="rxt", bufs=1) as rxt, tc.tile_pool(
            name="router_sbuf", bufs=2
        ) as spool:
            rpsum = psA
            xt32 = rxt.tile([128, ND * T], F32)
            nc.sync.dma_start(
                xt32[:].rearrange("p (a t) -> p a t", t=T),
                io["xT32"].ap().rearrange("(a p) t -> p a t", p=128),
            )
            for tt in range(NT):
                ps = rpsum.tile([128, E], F32, tag="psgl", name="rps")
                for dt in range(ND):
                    nc.tensor.matmul(
                        ps[:],
                        lhsT=xt32[:, dt * T + tt * 128: dt * T + (tt + 1) * 128],
                        rhs=wg_t[:, dt * E:(dt + 1) * E],
                        start=(dt == 0),
                        stop=False,
                    )
                nc.tensor.matmul(
                    ps[:], lhsT=ones32[:], rhs=bg_t[:], start=False, stop=True
                )
                logit = spool.tile([128, E], F32, tag="logit")
                nc.scalar.copy(logit[:], ps[:])
                m8 = spool.tile([128, 8], F32, tag="m8")
                nc.vector.max(m8[:], logit[:])
                negm = spool.tile([128, 1], F32, tag="negm")
                nc.vector.tensor_scalar_mul(negm[:], m8[:, 0:1], -1.0)
                p = spool.tile([128, E], F32, tag="p")
                nc.scalar.activation(p[:], logit[:], AF.Exp, bias=negm[:, 0:1])
                s = spool.tile([128, 1], F32, tag="s")
                nc.vector.reduce_sum(s[:], p[:], axis=mybir.AxisListType.X)
                r = spool.tile([128, 1], F32, tag="r")
                nc.vector.reciprocal(r[:], s[:])
                ksl = slice(tt * E, (tt + 1) * E)
                nc.vector.tensor_scalar(
                    keepT[:, ksl], logit[:], m8[:, 1:2], None, op0=ALU.is_ge
                )
                nc.vector.scalar_tensor_tensor(
                    out=g[:, ksl],
                    in0=p[:],
                    scalar=r[:, 0:1],
                    in1=keepT[:, ksl],
                    op0=ALU.mult,
                    op1=ALU.mult,
                )
                # rank: 0 for argmax expert, 1 for runner-up
                eq1 = spool.tile([128, E], F32, tag="eq1")
                nc.vector.tensor_scalar(
                    eq1[:], logit[:], m8[:, 0:1], None, op0=ALU.is_ge
                )
                rank = spool.tile([128, E], F32, tag="rank")
                nc.vector.tensor_tensor(
                    rank[:], keepT[:, ksl], eq1[:], op=ALU.subtract
                )
                # slotv = (rank*T + (1 + global_token)) * keep
                sv = spool.tile([128, E], F32, tag="sv")
                nc.vector.tensor_scalar(
                    sv[:],
                    rank[:],
                    float(T),
                    tgl_f[:, tt: tt + 1],
                    op0=ALU.mult,
                    op1=ALU.add,
                )
                nc.vector.tensor_tensor(
                    slotv[:, ksl], sv[:], keepT[:, ksl], op=ALU.mult
                )
            # exclusive prefix position of each kept token within its expert
            for tt in range(NT):
                ps = rpsum.tile([128, E], F32, tag="psgl", name="rps")
                for j in range(tt):
                    nc.tensor.matmul(
                        ps[:],
                        lhsT=ones128[:],
                        rhs=keepT[:, j * E:(j + 1) * E],
                        start=(j == 0),
                        stop=False,
                    )
                nc.tensor.matmul(
                    ps[:],
                    lhsT=lstrict[:],
                    rhs=keepT[:, tt * E:(tt + 1) * E],
                    start=(tt == 0),
                    stop=True,
                )
                nc.scalar.copy(pos[:, tt * E:(tt + 1) * E], ps[:])

        W1 = io["W1"].ap()
        W2 = io["W2"].ap()

        # ---- per-chunk sparse FFN ----
        for (e, s0, C) in chunks:
            JT = (C + 127) // 128
            ksl = lambda tt: slice(tt * E + e, tt * E + e + 1)  # noqa: E731

            # one-hot selection matrices P16 [t, j] (bf16); P32 built per-tt
            P16 = ppool.tile([128, NT * C], BF16, tag="P16")
            posc = mpool.tile([128, NT], F32, tag="posc")
            if s0:
                nc.vector.tensor_scalar(
                    posc[:], pos[:].rearrange("p (t e) -> p t e", e=E)[:, :, e],
                    float(s0), None, op0=ALU.subtract,
                )
            else:
                nc.vector.tensor_copy(
                    posc[:], pos[:].rearrange("p (t e) -> p t e", e=E)[:, :, e]
                )
            for tt in range(NT):
                nc.vector.tensor_scalar(
                    P16[:, tt * C:(tt + 1) * C],
                    iota_f[:, :C],
                    posc[:, tt: tt + 1],
                    keepT[:, ksl(tt)],
                    op0=ALU.is_equal,
                    op1=ALU.mult,
                )

            # gather gate + slot metadata: [C, 2] = P^T @ [g, slotv]
            g_sel = selpool.tile([128, JT], F32, tag="g_sel")
            slot_f = selpool.tile([128, JT], F32, tag="slot_f")
            nc.vector.memset(slot_f[:], 0.0)
            psms = [
                psB.tile([128, 2], F32, tag="ps2", name=f"psm_{jt}")
                for jt in range(JT)
            ]
            for tt in range(NT):
                p32t = mpool.tile([128, C], F32, tag="p32t", name=f"p32t_{tt}")
                nc.vector.tensor_scalar(
                    p32t[:],
                    iota_f[:, :C],
                    posc[:, tt: tt + 1],
                    keepT[:, ksl(tt)],
                    op0=ALU.is_equal,
                    op1=ALU.mult,
                )
                meta = mpool.tile([128, 2], F32, tag="meta", name=f"meta_{tt}")
                nc.gpsimd.tensor_copy(meta[:, 0:1], g[:, ksl(tt)])
                nc.gpsimd.tensor_copy(meta[:, 1:2], slotv[:, ksl(tt)])
                for jt in range(JT):
                    pj = min(128, C - jt * 128)
                    nc.tensor.matmul(
                        psms[jt][:pj, :],
                        lhsT=p32t[:, jt * 128: jt * 128 + pj],
                        rhs=meta[:],
                        start=(tt == 0),
                        stop=(tt == NT - 1),
                    )
            for jt in range(JT):
                pj = min(128, C - jt * 128)
                nc.scalar.copy(g_sel[:pj, jt: jt + 1], psms[jt][:pj, 0:1])
                nc.scalar.copy(slot_f[:pj, jt: jt + 1], psms[jt][:pj, 1:2])
            # adjust slots: kept -> slot-1 ; empty -> huge (dropped by bounds)
            emptymask = selpool.tile([128, JT], F32, tag="emptymask")
            nc.vector.tensor_scalar(
                emptymask[:], slot_f[:], 0.0, None, op0=ALU.is_equal
            )
            slot_adj = selpool.tile([128, JT], F32, tag="slot_adj")
            nc.vector.tensor_scalar(
                slot_adj[:], slot_f[:], 1.0, None, op0=ALU.subtract
            )
            slot_fin = selpool.tile([128, JT], F32, tag="slot_fin")
            nc.vector.scalar_tensor_tensor(
                out=slot_fin[:],
                in0=emptymask[:],
                scalar=float(2 * T + 2048),
                in1=slot_adj[:],
                op0=ALU.mult,
                op1=ALU.add,
            )
            slot_i = selpool.tile([128, JT], mybir.dt.int32, tag="slot_i")
            nc.vector.tensor_copy(slot_i[:], slot_fin[:])

            # gather selected tokens, transposed: xsel[d, j] = x^T @ P
            xsel = selpool.tile([128, ND * C], BF16, tag="xsel")
            for dt in range(ND):
                psg = psA.tile([128, C], F32, tag="psgl", name=f"psg_{dt}")
                for tt in range(NT):
                    nc.tensor.matmul(
                        psg[:],
                        lhsT=xtok[:, tt * D + dt * 128: tt * D + (dt + 1) * 128],
                        rhs=P16[:, tt * C:(tt + 1) * C],
                        start=(tt == 0),
                        stop=(tt == NT - 1),
                    )
                nc.scalar.copy(xsel[:, dt * C:(dt + 1) * C], psg[:])

            # layer 1: hTs[h, j] = gelu(W1^T xsel + b1)
            hTs = hpool.tile([128, NH * C], BF16, tag="hTs")
            for ht in range(NH):
                w1s = w1pool.tile([128, ND * 128], BF16, tag="w1s")
                nc.sync.dma_start(
                    w1s[:].rearrange("p (a j) -> p a j", j=128),
                    W1[e].rearrange("(a p) h -> p a h", p=128)[
                        :, :, ht * 128:(ht + 1) * 128
                    ],
                )
                ps1 = psA.tile([128, C], F32, tag="psgl", name=f"ps1_{ht}")
                for dt in range(ND):
                    nc.tensor.matmul(
                        ps1[:],
                        lhsT=w1s[:, dt * 128:(dt + 1) * 128],
                        rhs=xsel[:, dt * C:(dt + 1) * C],
                        start=(dt == 0),
                        stop=(dt == ND - 1),
                    )
                nc.scalar.activation(
                    hTs[:, ht * C:(ht + 1) * C],
                    ps1[:],
                    AF.Gelu,
                    bias=b1c_t[:, e * NH + ht: e * NH + ht + 1],
                )

            # layer 2 + gate + scatter
            b2e = b2pool.tile([1, D], BF16, tag="b2e")
            nc.sync.dma_start(b2e[:], io["b2r"].ap()[e])
            ps2 = [
                [
                    psB.tile(
                        [128, 512], F32, tag="ps2", name=f"ps2_{jt}_{db}"
                    )
                    for db in range(2)
                ]
                for jt in range(JT)
            ]
            for ht in range(NH):
                w2s = w2pool.tile([128, D], BF16, tag="w2s")
                nc.sync.dma_start(
                    w2s[:], W2[e][ht * 128:(ht + 1) * 128, :]
                )
                for jt in range(JT):
                    pj = min(128, C - jt * 128)
                    for db in range(2):
                        nc.tensor.matmul(
                            ps2[jt][db][:pj, :],
                            lhsT=hTs[:, ht * C + jt * 128: ht * C + jt * 128 + pj],
                            rhs=w2s[:, db * 512:(db + 1) * 512],
                            start=(ht == 0),
                            stop=False,
                        )
            for jt in range(JT):
                pj = min(128, C - jt * 128)
                eo = epool.tile([128, D], F32, tag="eo", name=f"eo_{jt}")
                for db in range(2):
                    nc.tensor.matmul(
                        ps2[jt][db][:pj, :],
                        lhsT=ones16[:, :pj],
                        rhs=b2e[:, db * 512:(db + 1) * 512],
                        start=False,
                        stop=True,
                    )
                    nc.vector.tensor_scalar_mul(
                        eo[:pj, db * 512:(db + 1) * 512],
                        ps2[jt][db][:pj, :],
                        g_sel[:pj, jt: jt + 1],
                    )
                nc.gpsimd.indirect_dma_start(
                    out=pairs.ap(),
                    out_offset=bass.IndirectOffsetOnAxis(
                        ap=slot_i[:pj, jt: jt + 1], axis=0
                    ),
                    in_=eo[:pj, :],
                    in_offset=None,
                    bounds_check=2 * T - 1,
                    oob_is_err=False,
                )

        # ---- combine (token, rank) pairs ----
        out = io["out"].ap()
        for mt in range(NT):
            pa = cpool.tile([128, D], F32, tag="pa")
            nc.sync.dma_start(pa[:], pairs.ap()[mt * 128:(mt + 1) * 128, :])
            pb = cpool.tile([128, D], F32, tag="pb")
            nc.sync.dma_start(pb[:], pairs.ap()[T + mt * 128: T + (mt + 1) * 128, :])
            nc.vector.tensor_add(pa[:], pa[:], pb[:])
            nc.sync.dma_start(out[mt * 128:(mt + 1) * 128, :], pa[:])


def _build_sparse(chunks):
    nc = bacc.Bacc(None, target_bir_lowering=False, debug=False, num_devices=N_CORES)
    io = {
        "xT32": nc.declare_dram_parameter("xT32", [D, T], F32, isOutput=False),
        "x16": nc.declare_dram_parameter("x16", [T, D], BF16, isOutput=False),
        "Wg": nc.declare_dram_parameter("Wg", [D, E], F32, isOutput=False),
        "bg": nc.declare_dram_parameter("bg", [1, E], F32, isOutput=False),
        "W1": nc.declare_dram_parameter("W1", [E, D, H], BF16, isOutput=False),
        "b1c": nc.declare_dram_parameter("b1c", [E, 128, NH], F32, isOutput=False),
        "W2": nc.declare_dram_parameter("W2", [E, H, D], BF16, isOutput=False),
        "b2r": nc.declare_dram_parameter("b2r", [E, 1, D], BF16, isOutput=False),
        "out": nc.declare_dram_parameter("out", [T, D], F32, isOutput=True),
    }
    with tile.TileContext(nc) as tc:
        _emit_sparse_moe(nc, tc, io, chunks)
    nc.compile()
    return nc


def _build_dense():
    nc = bacc.Bacc(None, target_bir_lowering=False, debug=False, num_devices=N_CORES)
    io = {
        "xT32": nc.declare_dram_parameter("xT32", [D, T], F32, isOutput=False),
        "xT16": nc.declare_dram_parameter("xT16", [D, T], BF16, isOutput=False),
        "Wg": nc.declare_dram_parameter("Wg", [D, E], F32, isOutput=False),
        "bg": nc.declare_dram_parameter("bg", [1, E], F32, isOutput=False),
        "W1": nc.declare_dram_parameter("W1", [E, D, H], BF16, isOutput=False),
        "b1c": nc.declare_dram_parameter("b1c", [E, 128, NH], F32, isOutput=False),
        "W2": nc.declare_dram_parameter("W2", [E, H, D], BF16, isOutput=False),
        "b2r": nc.declare_dram_parameter("b2r", [E, 1, D], BF16, isOutput=False),
        "out": nc.declare_dram_parameter("out", [T, D], F32, isOutput=True),
    }
    if DEBUG_ROUTER:
        nc.dbg_logits = nc.declare_dram_parameter("dbg_logits", [T, E], F32, isOutput=True)
    with tile.TileContext(nc) as tc:
        _emit_dense_moe(nc, tc, io)
    nc.compile()
    return nc


_CACHE = {}


def prep_inputs(x, Wg, bg, W1, b1, W2, b2):
    """Host-side shard + layout/dtype prep. Returns per-core input maps."""
    xt = np.ascontiguousarray(np.asarray(x, dtype=np.float32).reshape(-1, D))
    Wg = np.asarray(Wg, dtype=np.float32)
    bg = np.asarray(bg, dtype=np.float32).reshape(1, E)
    W1b = np.asarray(W1, dtype=np.float32).astype(BFNP)
    W2b = np.asarray(W2, dtype=np.float32).astype(BFNP)
    b1c = np.ascontiguousarray(
        np.asarray(b1, dtype=np.float32).reshape(E, NH, 128).transpose(0, 2, 1)
    )
    b2r = np.asarray(b2, dtype=np.float32).astype(BFNP).reshape(E, 1, D)

    in_maps = []
    for c in range(N_CORES):
        xs = xt[c * T:(c + 1) * T]           # [T, D]
        xT32 = np.ascontiguousarray(xs.T)    # [D, T]
        in_maps.append(
            {
                "xT32": xT32,
                "xT16": xT32.astype(BFNP),
                "x16": xs.astype(BFNP),
                "Wg": Wg,
                "bg": bg,
                "W1": W1b,
                "b1c": b1c,
                "W2": W2b,
                "b2r": b2r,
            }
        )
    return in_maps


MODE = "sparse"


def kernel(x, Wg, bg, W1, b1, W2, b2):
    B_, S_, D_ = x.shape
    in_maps = prep_inputs(x, Wg, bg, W1, b1, W2, b2)
    if MODE == "sparse":
        chunks = route_capacities({"x": x, "Wg": Wg, "bg": bg})
        key = ("sparse", chunks)
        if key not in _CACHE:
            _CACHE[key] = _build_sparse(chunks)
    else:
        key = "dense"
        if key not in _CACHE:
            _CACHE[key] = _build_dense()
    nc = _CACHE[key]
    res = run_bass_kernel_spmd(nc, in_maps, list(range(N_CORES)))
    out = np.concatenate([res.results[c]["out"] for c in range(N_CORES)], axis=0)
    return out.reshape(B_, S_, D_).astype(np.float32)


if __name__ == "__main__":
    os.environ.setdefault("JAX_PLATFORMS", "")
    sys.path.insert(0, "/root/problem")
    import reference as R

    inputs = {k: np.asarray(v) for k, v in R.setup_inputs().items()}
    got = kernel(**inputs)
    import jax

    with jax.default_device(jax.devices("cpu")[0]):
        want = np.asarray(R.reference(**{k: np.asarray(v) for k, v in inputs.items()}))
    diff = np.abs(got - want)
    scale = np.abs(want).max()
    rel_fro = np.linalg.norm(diff) / np.linalg.norm(want)
    print(f"absmax err: {diff.max():.3e}  scale: {scale:.3e}  "
          f"absmax/scale: {diff.max() / scale:.3e}  rel_fro: {rel_fro:.3e}")
